# revision 33
# baseline (speedup 1.0000x reference)
# Self-contained Trainium2 Bass kernel for nn_MultiInputLSTMCell.
#
# Reference computation (all fp32):
#   pre   = h0 @ W_hh + bias + input_ @ W_ih          # (1, 3H)
#   i, o  = sigmoid(pre[:, :H]), sigmoid(pre[:, H:2H])
#   g     = tanh(pre[:, 2H:])
#   awi   = input_ @ aW_ih + a_bias                   # (1, H)
#   awh   = c_input @ aW_hh                           # (C, H)
#   alpha = sigmoid(awi + awh)                        # (C, H)
#   w     = exp([i; alpha]); w /= w.sum(0)            # (C+1, H)
#   c1    = (([g; c_input]) * w).sum(0)               # (1, H)
#   h1    = o * tanh(c1)
#
# Strategy: tensor-parallel over the hidden (output-column) dim across 8
# cores (HS = 256 columns each); all post-matmul work is local to a shard.
#
# Performance design (baseline bf16 45.5us -> this kernel):
#  * All weights ship as fp8 e3m4 (4 mantissa bits) at a power-of-2 scale
#    (w*64; g columns w*128 with the tanh(x)=2*sigmoid(2x)-1 factor folded
#    in), halving HBM traffic vs bf16 to ~4.5 MB/core.  The scale is undone
#    by the ACT activation `scale` (-1/64), costing zero extra ops.  Host
#    numpy model of this quantization: 8.7e-3 absmax-rel error (gate 2e-2).
#  * The HAM activity throttle runs the PE at half clock until ~8.5us of
#    sustained activity: warm-up matmuls start the integrator at t=0, and
#    the [i|g] gates / alpha_wi GEMV streams are 4-way column-tiled
#    (tile_position (0,32j), partial sums at PSUM partitions 0/32/64/96)
#    so even a cold PE outruns the DMA.  Partials are summed by a masked
#    K=97 ones-matmul off the critical path (for alpha_wi the combine is
#    fused into the existing broadcast matmul).
#  * The o-gate weight columns stream LAST: the softmax reduction / c1
#    tail overlaps the o-column DMA+matmuls.  The i/g gate tail uses one
#    native TANH (exp and tanh coexist in the ACT table - no reload;
#    native Sigmoid DOES reload, ~1.3us) via sigmoid(x)=(1+tanh(x/2))/2
#    and exp(sigmoid(pre_i)) = e^0.5 * exp(0.5*tanh(pre_i/2)).
#  * Dual HWDGE rings: the scalar (ACT) ring (~90GB/s) carries ct + the
#    alpha_hh weights concurrently with the sync ring (~265GB/s) carrying
#    the gates/o/alpha_ih stream - measured +25% aggregate bandwidth.

import numpy as np

import concourse.bass as bass
import concourse.tile as tile
from concourse import bacc, mybir
from concourse.bass_utils import run_bass_kernel_spmd

NCORES = 8
H = 2048          # hidden size
IN = 2048         # input size
C = 64            # number of skip-word cell states
HS = H // NCORES  # hidden shard per core = 256
KG = IN + H       # gates contraction dim = 4096
KO_G = KG // 128  # 32 k-chunks for gates
KO_A = IN // 128  # 16 k-chunks per alpha matmul
F32 = mybir.dt.float32
F32R = mybir.dt.float32r
BF16 = mybir.dt.bfloat16
FP8 = mybir.dt.float8e3   # e3m4: 4 mantissa bits, max +-15.5
WSCALE = 64.0             # uniform PSUM scale for quantized weights

_nc_cache = None


def _build_nc():
    """Build the single-core Bass program (same program runs on all 8 cores)."""
    nc = bacc.Bacc(
        "TRN2",
        target_bir_lowering=False,
        debug=False,
        enable_asserts=False,
        name="multi_input_lstm_cell",
    )

    # DRAM I/O (per-core shards; shapes identical on every core).
    # Weights are host-pre-tiled to [ki=128, ko, n] so each chunk DMA reads
    # one long contiguous segment per partition at full HBM efficiency.
    wig = nc.dram_tensor("wig", [128, KO_G, 2 * HS], FP8, kind="ExternalInput").ap()
    wo = nc.dram_tensor("wo", [128, KO_G, HS], FP8, kind="ExternalInput").ap()
    # wa rows 0..2047 = alpha_weight_ih shard, rows 2048..4095 = alpha_weight_hh
    wa = nc.dram_tensor("wa", [128, 2 * KO_A, HS], FP8, kind="ExternalInput").ap()
    # bab = [b_i*64 | b_g*128 | b_o*64 | ab*64]
    bab = nc.dram_tensor("bab", [1, 4 * HS], F32, kind="ExternalInput").ap()
    cs = nc.dram_tensor("cs", [C, HS], F32R, kind="ExternalInput").ap()
    ones1 = nc.dram_tensor("ones1", [C, 1], F32R, kind="ExternalInput").ap()
    # mask4[k, m] = 1.0 iff k in {0,32,64,96}: sums 4 column-tiled partial
    # rows (and, with m>1, broadcasts the sum to m output partitions)
    mask4 = nc.dram_tensor("mask4", [97, C], F32R, kind="ExternalInput").ap()
    xt = nc.dram_tensor("xt", [128, KO_G], BF16, kind="ExternalInput").ap()
    ct = nc.dram_tensor("ct", [128, KO_A, C], BF16, kind="ExternalInput").ap()
    # hc[0, 0:256] = c1 shard, hc[0, 256:512] = h1 shard
    hc = nc.dram_tensor("hc", [1, 2 * HS], F32, kind="ExternalOutput").ap()

    with tile.TileContext(nc) as tc:
        _emit(tc, wig, wo, wa, bab, cs, ones1, mask4, xt, ct, hc)

    nc.compile()
    return nc


def _emit(tc, wig, wo, wa, bab, cs, ones1, mask4, xt, ct, hc):
    from contextlib import ExitStack

    nc = tc.nc
    TANH = mybir.ActivationFunctionType.Tanh
    EXP = mybir.ActivationFunctionType.Exp
    INV_S = 1.0 / WSCALE
    E_HALF = 1.6487212707001282  # exp(0.5)

    with ExitStack() as ctx:
        singles = ctx.enter_context(tc.tile_pool(name="singles", bufs=1))
        wig_pool = ctx.enter_context(tc.tile_pool(name="wig_pool", bufs=4))
        wo_pool = ctx.enter_context(tc.tile_pool(name="wo_pool", bufs=3))
        psum = ctx.enter_context(tc.tile_pool(name="psum", bufs=1, space="PSUM"))

        xt_t = singles.tile([128, KO_G], BF16, tag="xt")
        bab_t = singles.tile([1, 4 * HS], F32, tag="bab")
        ew_t = singles.tile([C, HS], F32R, tag="ew")
        mg_t = singles.tile([C, HS], F32R, tag="mg")
        ones_r = singles.tile([C, 1], F32R, tag="ones_r")
        mask4_t = singles.tile([97, C], F32R, tag="mask4")
        ones_b = singles.tile([1, 1], F32, tag="ones_b")
        nc.vector.memset(ones_b[:], 1.0)
        wz_l = singles.tile([128, 97], BF16, tag="wz_l")
        nc.vector.memset(wz_l[:], 0.0)
        wz_r = singles.tile([128, 512], BF16, tag="wz_r")
        nc.vector.memset(wz_r[:], 0.0)
        warm_t = singles.tile([1, 1], F32, tag="warm")
        nc.vector.memset(warm_t[:], 0.0)
        nc.scalar.activation(out=warm_t[:], in_=warm_t[:], func=EXP)

        wa_t = singles.tile([128, 2 * KO_A, HS], FP8, tag="wa")
        ct_t = singles.tile([128, KO_A, C], BF16, tag="ct")

        pgig4 = psum.tile([97, 512], F32, tag="pgig4")   # [i|g] 4 partials
        pgwi4 = psum.tile([97, HS], F32, tag="pgwi4")    # alpha_wi 4 partials
        pgo4 = psum.tile([97, HS], F32, tag="pgo4")      # o gate 4 partials
        pal = psum.tile([C, HS], F32, tag="pal")         # alpha pre-activation
        pgig = psum.tile([1, 512], F32, tag="pgig")      # combined [i|g]
        pgo = psum.tile([1, HS], F32, tag="pgo")         # combined o
        ps0 = psum.tile([1, HS], F32, tag="ps0")
        ps1 = psum.tile([1, HS], F32, tag="ps1")

        # ---- sync ring: xt/bab, alpha_wi half, gates [i|g], o last ----
        nc.sync.dma_start(out=xt_t[:], in_=xt)
        nc.sync.dma_start(out=bab_t[:], in_=bab)
        IG_SIZES = [4, 4, 12, 12]
        ig_starts = [sum(IG_SIZES[:i]) for i in range(len(IG_SIZES))]
        O_SIZES = [12, 12, 8]
        o_starts = [sum(O_SIZES[:i]) for i in range(len(O_SIZES))]
        ig_tiles = []
        for ci, sz in enumerate(IG_SIZES):
            t = wig_pool.tile([128, 12, 2 * HS], FP8, tag="wig")
            nc.sync.dma_start(
                out=t[:, 0:sz, :], in_=wig[:, ig_starts[ci] : ig_starts[ci] + sz, :]
            )
            ig_tiles.append(t)
            if ci == 0:
                # alpha_wi half rides after the first gates tile so the PE
                # gets real work ~3us earlier (two slices: the wi matmuls
                # sit early in the in-order PE queue and must not stall on
                # one big transfer)
                for lo in (0, 8):
                    nc.sync.dma_start(out=wa_t[:, lo : lo + 8, :],
                                      in_=wa[:, lo : lo + 8, :])
        o_tiles = []
        for ci, sz in enumerate(O_SIZES):
            t = wo_pool.tile([128, 12, HS], FP8, tag="wo")
            nc.sync.dma_start(
                out=t[:, 0:sz, :], in_=wo[:, o_starts[ci] : o_starts[ci] + sz, :]
            )
            o_tiles.append(t)

        # ---- scalar ring (slow ~90GB/s, runs concurrently): ct + the
        # alpha_hh half + small tensors; all consumed mid-kernel.  Moving
        # more than ~0.9MB here LOWERS aggregate bandwidth (the rings share
        # the 16 SDMA engines; measured 260 vs 330GB/s at a 1.4MB share).
        nc.scalar.dma_start(out=ct_t[:], in_=ct)
        nc.scalar.dma_start(out=wa_t[:, 16:24, :], in_=wa[:, 16:24, :])
        nc.scalar.dma_start(out=mask4_t[:], in_=mask4)
        nc.scalar.dma_start(out=mg_t[:], in_=cs)
        nc.scalar.dma_start(out=wa_t[:, 24:32, :], in_=wa[:, 24:32, :])
        nc.scalar.dma_start(out=ones_r[:], in_=ones1)

        # ---- PE helpers ----------------------------------------------
        def ig_mms(lo, hi):
            # chunks 0..27 go to the 4-way partials (groups close at 24..27);
            # chunks 28..31 accumulate directly into the combined pgig after
            # the masked combine ran, so the copy+combine overlap them.
            for kk in range(lo, hi):
                ci = max(i for i, s in enumerate(ig_starts) if s <= kk)
                j = kk % 4
                nc.tensor.matmul(
                    pgig4[32 * j : 32 * j + 1, :],
                    lhsT=xt_t[:, kk : kk + 1],
                    rhs=ig_tiles[ci][:, kk - ig_starts[ci], :],
                    start=False,
                    stop=(24 <= kk < 28),
                    tile_position=(0, 32 * j),
                    skip_group_check=True,
                )

        def wi_mms(lo, hi):
            for ko in range(lo, hi):
                j = ko % 4
                nc.tensor.matmul(
                    pgwi4[32 * j : 32 * j + 1, :],
                    lhsT=xt_t[:, KO_A + ko : KO_A + ko + 1],
                    rhs=wa_t[:, ko, :],
                    start=False,
                    stop=(ko >= KO_A - 4),
                    tile_position=(0, 32 * j),
                    skip_group_check=True,
                )

        def ahh_mms(lo, hi):
            for ko in range(lo, hi):
                nc.tensor.matmul(
                    pal[:],
                    lhsT=ct_t[:, ko, :],
                    rhs=wa_t[:, KO_A + ko, :],
                    start=(ko == 0),
                    stop=False,
                )

        def o_mms(lo, hi):
            # chunks 0..27 go to the partials (groups close at 24..27);
            # 28..31 accumulate into the combined pgo so the copy+combine
            # overlap them instead of serializing after the last matmul.
            for kk in range(lo, hi):
                ci = max(i for i, s in enumerate(o_starts) if s <= kk)
                j = kk % 4
                nc.tensor.matmul(
                    pgo4[32 * j : 32 * j + 1, :],
                    lhsT=xt_t[:, kk : kk + 1],
                    rhs=o_tiles[ci][:, kk - o_starts[ci], :],
                    start=False,
                    stop=(24 <= kk < 28),
                    tile_position=(0, 32 * j),
                    skip_group_check=True,
                )

        # ---- PE emission (matches data-arrival order) ----------------
        nc.tensor.matmul(pgig4[:], lhsT=wz_l[:], rhs=wz_r[:],
                         start=True, stop=True, skip_group_check=True)
        nc.tensor.matmul(pgig4[0:1, :], lhsT=ones_b[:], rhs=bab_t[:, 0:512],
                         start=False, stop=False, tile_position=(0, 0),
                         skip_group_check=True)
        nc.tensor.matmul(pgwi4[:], lhsT=wz_l[:], rhs=wz_r[:, 0:HS],
                         start=True, stop=True, skip_group_check=True)
        nc.tensor.matmul(pgo4[:], lhsT=wz_l[:], rhs=wz_r[:, 0:HS],
                         start=True, stop=True, skip_group_check=True)
        nc.tensor.matmul(pgwi4[0:1, :], lhsT=ones_b[:], rhs=bab_t[:, 768:1024],
                         start=False, stop=False, tile_position=(0, 0),
                         skip_group_check=True)
        nc.tensor.matmul(pgo4[0:1, :], lhsT=ones_b[:], rhs=bab_t[:, 512:768],
                         start=False, stop=False, tile_position=(0, 0),
                         skip_group_check=True)
        # extra warm-up matmuls: keep the HAM activity integrator running
        # while the first weight tiles are still in flight (pgig is reset
        # by the combine matmul's start=True later)
        for _ in range(4):
            nc.tensor.matmul(pgig[:], lhsT=wz_l[:, 0:1], rhs=wz_r[:],
                             start=True, stop=True, skip_group_check=True)
        ig_mms(0, 4)
        wi_mms(0, 8)
        ig_mms(4, 8)
        wi_mms(8, KO_A)
        ahh_mms(0, 8)
        ig_mms(8, 16)
        ahh_mms(8, KO_A)

        # wi partials -> SBUF (ACT), masked broadcast-sum into pal
        wi4_t = singles.tile([97, HS], F32R, tag="wi4")
        nc.scalar.copy(out=wi4_t[:], in_=pgwi4[:])
        nc.tensor.matmul(
            pal[:], lhsT=mask4_t[:, 0:C], rhs=wi4_t[:], start=False, stop=True,
        )

        ig_mms(16, 28)

        # [i|g] partials combine (overlaps the last four gates chunks)
        ig4_t = singles.tile([97, 512], F32R, tag="ig4")
        nc.scalar.copy(out=ig4_t[:], in_=pgig4[:])
        nc.tensor.matmul(pgig[:], lhsT=mask4_t[:, 0:1], rhs=ig4_t[:],
                         start=True, stop=False)
        for kk in range(28, KO_G):
            ci = max(i for i, s in enumerate(ig_starts) if s <= kk)
            nc.tensor.matmul(
                pgig[:],
                lhsT=xt_t[:, kk : kk + 1],
                rhs=ig_tiles[ci][:, kk - ig_starts[ci], :],
                start=False,
                stop=(kk == KO_G - 1),
                skip_group_check=True,
            )

        # ---- alpha rows tail (ACT/DVE; overlaps the o matmuls) --------
        tmp_a = singles.tile([C, HS], F32, tag="tmp_a")
        nc.scalar.activation(out=tmp_a[:], in_=pal[:], func=EXP, scale=-INV_S)
        nc.vector.tensor_scalar_add(out=tmp_a[:], in0=tmp_a[:], scalar1=1.0)
        nc.vector.reciprocal_approx_fast(out=tmp_a[:], in_=tmp_a[:])
        nc.scalar.activation(out=ew_t[:], in_=tmp_a[:], func=EXP)
        nc.vector.tensor_mul(out=mg_t[:], in0=mg_t[:], in1=ew_t[:])

        # ---- gates [i|g] tail: one native TANH covers both gates ------
        #   th = [tanh(pre_i/2) | tanh(pre_g)]   (one scale 1/128)
        #   exp(sigmoid(pre_i)) = e^0.5 * exp(0.5*th_i);  g = th_g
        th_t = singles.tile([1, 512], F32, tag="th")
        nc.scalar.activation(out=th_t[:], in_=pgig[:], func=TANH, scale=0.5 * INV_S)
        ew64_t = singles.tile([1, HS], F32, tag="ew64")
        nc.scalar.activation(out=ew64_t[:], in_=th_t[:, 0:HS], func=EXP, scale=0.5)
        mg64_t = singles.tile([1, HS], F32, tag="mg64")
        nc.vector.scalar_tensor_tensor(
            out=mg64_t[:], in0=ew64_t[:], scalar=E_HALF, in1=th_t[:, HS:512],
            op0=mybir.AluOpType.mult, op1=mybir.AluOpType.mult)

        o_mms(0, 24)
        # K=64 reductions over the alpha rows (emitted after their inputs'
        # writers - Tile dependency tracking is program-order-based)
        nc.tensor.matmul(ps0[:], lhsT=ones_r[:], rhs=ew_t[:],
                         start=True, stop=True)
        nc.tensor.matmul(ps1[:], lhsT=ones_r[:], rhs=mg_t[:],
                         start=True, stop=True)
        o_mms(24, 28)

        # o partials combine (overlaps the last four o chunks)
        o4_t = singles.tile([97, HS], F32R, tag="o4")
        nc.scalar.copy(out=o4_t[:], in_=pgo4[:])
        nc.tensor.matmul(pgo[:], lhsT=mask4_t[:, 0:1], rhs=o4_t[:],
                         start=True, stop=False)
        for kk in range(28, KO_G):
            ci = max(i for i, s in enumerate(o_starts) if s <= kk)
            nc.tensor.matmul(
                pgo[:],
                lhsT=xt_t[:, kk : kk + 1],
                rhs=o_tiles[ci][:, kk - o_starts[ci], :],
                start=False,
                stop=(kk == KO_G - 1),
                skip_group_check=True,
            )

        # ---- close the softmax with the i/g row on DVE ----------------
        s0_t = singles.tile([1, HS], F32, tag="s0")
        nc.vector.scalar_tensor_tensor(
            out=s0_t[:], in0=ew64_t[:], scalar=E_HALF, in1=ps0[:],
            op0=mybir.AluOpType.mult, op1=mybir.AluOpType.add)
        s1_t = singles.tile([1, HS], F32, tag="s1")
        nc.vector.tensor_add(out=s1_t[:], in0=ps1[:], in1=mg64_t[:])
        r_t = singles.tile([1, HS], F32, tag="r")
        nc.vector.reciprocal_approx_fast(out=r_t[:], in_=s0_t[:])
        hc_t = singles.tile([1, 2 * HS], F32, tag="hc")
        c1_t = hc_t[:, 0:HS]
        nc.vector.tensor_mul(out=c1_t, in0=s1_t[:], in1=r_t[:])
        nc.sync.dma_start(out=hc[:, 0:HS], in_=c1_t)

        # h1 = tanh(c1) / (1 + exp(-pre_o)); exp/tanh coexist in the ACT
        # table so neither reloads.
        oe_t = singles.tile([1, HS], F32, tag="oe")
        nc.scalar.activation(out=oe_t[:], in_=pgo[:], func=EXP, scale=-INV_S)
        nc.vector.tensor_scalar_add(out=oe_t[:], in0=oe_t[:], scalar1=1.0)
        nc.vector.reciprocal_approx_fast(out=oe_t[:], in_=oe_t[:])
        t4_t = singles.tile([1, HS], F32, tag="t4")
        nc.scalar.activation(out=t4_t[:], in_=c1_t, func=TANH)
        nc.vector.tensor_mul(out=hc_t[:, HS : 2 * HS], in0=oe_t[:], in1=t4_t[:])

        nc.sync.dma_start(out=hc[:, HS : 2 * HS], in_=hc_t[:, HS : 2 * HS])


def _shard_inputs(input_, c_input, h0, c0, weight_ih, weight_hh,
                  alpha_weight_ih, alpha_weight_hh, bias, alpha_bias):
    """Host-side scatter: column-shard the weights over the hidden dim.

    Weights are quantized once to fp8 e3m4 at scale 64 (g columns 128, the
    tanh 2x factor folded in) and pre-tiled to the [ki=128, ko, n] SBUF
    layout; per-core shards are then cheap slices.
    """
    import ml_dtypes
    f32 = np.float32
    bf16 = ml_dtypes.bfloat16
    e3m4 = ml_dtypes.float8_e3m4

    x_comb = np.concatenate([h0[0], input_[0]]).astype(f32)          # (4096,)
    xt = np.ascontiguousarray(x_comb.reshape(KO_G, 128).T).astype(bf16)
    # c_input.T tiled to [ki=128, ko=16, C]
    ct = np.ascontiguousarray(
        c_input.T.reshape(KO_A, 128, C).transpose(1, 0, 2)).astype(bf16)

    w_full = np.concatenate([weight_hh, weight_ih], axis=0).astype(f32)  # (4096, 3H)
    wig_full = np.empty((KG, 2 * H), f32)
    wig_full[:, 0:H] = w_full[:, 0:H] * WSCALE              # i columns
    wig_full[:, H : 2 * H] = w_full[:, 2 * H : 3 * H] * (2.0 * WSCALE)  # g columns
    wo_full = w_full[:, H : 2 * H] * WSCALE                 # o columns
    del w_full
    wig_t = np.ascontiguousarray(
        wig_full.astype(e3m4).reshape(KO_G, 128, 2 * H).transpose(1, 0, 2))
    del wig_full
    wo_t = np.ascontiguousarray(
        wo_full.astype(e3m4).reshape(KO_G, 128, H).transpose(1, 0, 2))
    del wo_full

    wa_full = np.concatenate([alpha_weight_ih, alpha_weight_hh], axis=0) * WSCALE
    wa_t = np.ascontiguousarray(
        wa_full.astype(e3m4).reshape(2 * KO_A, 128, H).transpose(1, 0, 2))
    del wa_full

    bias = np.asarray(bias, f32)
    alpha_bias = np.asarray(alpha_bias, f32)
    c_input = np.asarray(c_input, f32)

    mask4 = np.zeros((97, C), f32)
    mask4[0::32, :] = 1.0

    in_maps = []
    for k in range(NCORES):
        cols = np.s_[k * HS : (k + 1) * HS]
        wig_k = np.ascontiguousarray(np.concatenate(
            [wig_t[:, :, 0 * H + k * HS : 0 * H + (k + 1) * HS],
             wig_t[:, :, 1 * H + k * HS : 1 * H + (k + 1) * HS]], axis=2))
        bab = np.concatenate(
            [bias[0 * H + k * HS : 0 * H + (k + 1) * HS] * WSCALE,
             bias[2 * H + k * HS : 2 * H + (k + 1) * HS] * (2.0 * WSCALE),
             bias[1 * H + k * HS : 1 * H + (k + 1) * HS] * WSCALE,
             alpha_bias[cols] * WSCALE])[None, :].astype(f32)
        in_maps.append({
            "wig": wig_k,
            "wo": np.ascontiguousarray(wo_t[:, :, cols]),
            "wa": np.ascontiguousarray(wa_t[:, :, cols]),
            "bab": bab,
            "cs": np.ascontiguousarray(c_input[:, cols]),
            "ones1": np.ones((C, 1), f32),
            "mask4": mask4,
            "xt": xt,
            "ct": ct,
        })
    return in_maps


def _run(inputs, trace=False):
    global _nc_cache
    if _nc_cache is None:
        _nc_cache = _build_nc()
    nc = _nc_cache
    in_maps = _shard_inputs(**inputs)
    res = run_bass_kernel_spmd(nc, in_maps, core_ids=list(range(NCORES)), trace=trace)
    h1 = np.concatenate(
        [res.results[k]["hc"][:, HS : 2 * HS] for k in range(NCORES)], axis=1)
    c1 = np.concatenate(
        [res.results[k]["hc"][:, 0:HS] for k in range(NCORES)], axis=1)
    return (h1.astype(np.float32), c1.astype(np.float32)), res


def kernel(input_, c_input, h0, c0, weight_ih, weight_hh,
           alpha_weight_ih, alpha_weight_hh, bias, alpha_bias):
    inputs = dict(
        input_=np.asarray(input_, np.float32),
        c_input=np.asarray(c_input, np.float32),
        h0=np.asarray(h0, np.float32),
        c0=np.asarray(c0, np.float32),
        weight_ih=np.asarray(weight_ih, np.float32),
        weight_hh=np.asarray(weight_hh, np.float32),
        alpha_weight_ih=np.asarray(alpha_weight_ih, np.float32),
        alpha_weight_hh=np.asarray(alpha_weight_hh, np.float32),
        bias=np.asarray(bias, np.float32),
        alpha_bias=np.asarray(alpha_bias, np.float32),
    )
    out, _ = _run(inputs)
    return out


# revision 34
# speedup vs baseline: 1.0585x; 1.0585x over previous
# Self-contained Trainium2 Bass kernel for nn_MultiInputLSTMCell.
#
# Reference computation (all fp32):
#   pre   = h0 @ W_hh + bias + input_ @ W_ih          # (1, 3H)
#   i, o  = sigmoid(pre[:, :H]), sigmoid(pre[:, H:2H])
#   g     = tanh(pre[:, 2H:])
#   awi   = input_ @ aW_ih + a_bias                   # (1, H)
#   awh   = c_input @ aW_hh                           # (C, H)
#   alpha = sigmoid(awi + awh)                        # (C, H)
#   w     = exp([i; alpha]); w /= w.sum(0)            # (C+1, H)
#   c1    = (([g; c_input]) * w).sum(0)               # (1, H)
#   h1    = o * tanh(c1)
#
# Strategy: tensor-parallel over the hidden (output-column) dim across 8
# cores (HS = 256 columns each); all post-matmul work is local to a shard.
#
# Performance design (baseline bf16 45.5us -> this kernel):
#  * All weights ship as fp8 e3m4 (4 mantissa bits) at a power-of-2 scale
#    (w*64; g columns w*128 with the tanh(x)=2*sigmoid(2x)-1 factor folded
#    in), halving HBM traffic vs bf16 to ~4.5 MB/core.  The scale is undone
#    by the ACT activation `scale` (-1/64), costing zero extra ops.  Host
#    numpy model of this quantization: 8.7e-3 absmax-rel error (gate 2e-2).
#  * The HAM activity throttle runs the PE at half clock until ~8.5us of
#    sustained activity: warm-up matmuls start the integrator at t=0, and
#    the [i|g] gates / alpha_wi GEMV streams are 4-way column-tiled
#    (tile_position (0,32j), partial sums at PSUM partitions 0/32/64/96)
#    so even a cold PE outruns the DMA.  Partials are summed by a masked
#    K=97 ones-matmul off the critical path (for alpha_wi the combine is
#    fused into the existing broadcast matmul).
#  * The o-gate weight columns stream LAST: the softmax reduction / c1
#    tail overlaps the o-column DMA+matmuls.  The i/g gate tail uses one
#    native TANH (exp and tanh coexist in the ACT table - no reload;
#    native Sigmoid DOES reload, ~1.3us) via sigmoid(x)=(1+tanh(x/2))/2
#    and exp(sigmoid(pre_i)) = e^0.5 * exp(0.5*tanh(pre_i/2)).
#  * Dual HWDGE rings: the scalar (ACT) ring (~90GB/s) carries ct + the
#    alpha_hh weights concurrently with the sync ring (~265GB/s) carrying
#    the gates/o/alpha_ih stream - measured +25% aggregate bandwidth.

import numpy as np

import concourse.bass as bass
import concourse.tile as tile
from concourse import bacc, mybir
from concourse.bass_utils import run_bass_kernel_spmd

NCORES = 8
H = 2048          # hidden size
IN = 2048         # input size
C = 64            # number of skip-word cell states
HS = H // NCORES  # hidden shard per core = 256
KG = IN + H       # gates contraction dim = 4096
KO_G = KG // 128  # 32 k-chunks for gates
KO_A = IN // 128  # 16 k-chunks per alpha matmul
F32 = mybir.dt.float32
F32R = mybir.dt.float32r
BF16 = mybir.dt.bfloat16
FP8 = mybir.dt.float8e3   # e3m4: 4 mantissa bits, max +-15.5
WSCALE = 64.0             # uniform PSUM scale for quantized weights

_nc_cache = None


def _build_nc():
    """Build the single-core Bass program (same program runs on all 8 cores)."""
    nc = bacc.Bacc(
        "TRN2",
        target_bir_lowering=False,
        debug=False,
        enable_asserts=False,
        name="multi_input_lstm_cell",
    )

    # DRAM I/O (per-core shards; shapes identical on every core).
    # Weights are host-pre-tiled to [ki=128, ko, n] so each chunk DMA reads
    # one long contiguous segment per partition at full HBM efficiency.
    wig = nc.dram_tensor("wig", [128, KO_G, 2 * HS], FP8, kind="ExternalInput").ap()
    wo = nc.dram_tensor("wo", [128, KO_G, HS], FP8, kind="ExternalInput").ap()
    # wa rows 0..2047 = alpha_weight_ih shard, rows 2048..4095 = alpha_weight_hh
    wa = nc.dram_tensor("wa", [128, 2 * KO_A, HS], FP8, kind="ExternalInput").ap()
    # bab = [b_i*64 | b_g*128 | b_o*64 | ab*64]
    bab = nc.dram_tensor("bab", [1, 4 * HS], F32, kind="ExternalInput").ap()
    cs = nc.dram_tensor("cs", [C, HS], F32R, kind="ExternalInput").ap()
    ones1 = nc.dram_tensor("ones1", [C, 1], F32R, kind="ExternalInput").ap()
    # mask4[k, m] = 1.0 iff k in {0,32,64,96}: sums 4 column-tiled partial
    # rows (and, with m>1, broadcasts the sum to m output partitions)
    mask4 = nc.dram_tensor("mask4", [97, C], F32R, kind="ExternalInput").ap()
    xt = nc.dram_tensor("xt", [128, KO_G], BF16, kind="ExternalInput").ap()
    ct = nc.dram_tensor("ct", [128, KO_A, C], BF16, kind="ExternalInput").ap()
    # hc[0, 0:256] = c1 shard, hc[0, 256:512] = h1 shard
    hc = nc.dram_tensor("hc", [1, 2 * HS], F32, kind="ExternalOutput").ap()

    with tile.TileContext(nc) as tc:
        _emit(tc, wig, wo, wa, bab, cs, ones1, mask4, xt, ct, hc)

    nc.compile()
    return nc


def _emit(tc, wig, wo, wa, bab, cs, ones1, mask4, xt, ct, hc):
    from contextlib import ExitStack

    nc = tc.nc
    TANH = mybir.ActivationFunctionType.Tanh
    EXP = mybir.ActivationFunctionType.Exp
    INV_S = 1.0 / WSCALE
    E_HALF = 1.6487212707001282  # exp(0.5)

    with ExitStack() as ctx:
        singles = ctx.enter_context(tc.tile_pool(name="singles", bufs=1))
        wig_pool = ctx.enter_context(tc.tile_pool(name="wig_pool", bufs=6))
        wo_pool = ctx.enter_context(tc.tile_pool(name="wo_pool", bufs=4))
        psum = ctx.enter_context(tc.tile_pool(name="psum", bufs=1, space="PSUM"))

        xt_t = singles.tile([128, KO_G], BF16, tag="xt")
        bab_t = singles.tile([1, 4 * HS], F32, tag="bab")
        ew_t = singles.tile([C, HS], F32R, tag="ew")
        mg_t = singles.tile([C, HS], F32R, tag="mg")
        ones_r = singles.tile([C, 1], F32R, tag="ones_r")
        mask4_t = singles.tile([97, C], F32R, tag="mask4")
        ones_b = singles.tile([1, 1], F32, tag="ones_b")
        nc.vector.memset(ones_b[:], 1.0)
        wz_l = singles.tile([128, 97], BF16, tag="wz_l")
        nc.vector.memset(wz_l[:], 0.0)
        wz_r = singles.tile([128, 512], BF16, tag="wz_r")
        nc.vector.memset(wz_r[:], 0.0)
        warm_t = singles.tile([1, 1], F32, tag="warm")
        nc.vector.memset(warm_t[:], 0.0)
        nc.scalar.activation(out=warm_t[:], in_=warm_t[:], func=EXP)

        wa_t = singles.tile([128, 2 * KO_A, HS], FP8, tag="wa")
        ct_t = singles.tile([128, KO_A, C], BF16, tag="ct")

        pgig4 = psum.tile([97, 512], F32, tag="pgig4")   # [i|g] 4 partials
        pgwi4 = psum.tile([97, HS], F32, tag="pgwi4")    # alpha_wi 4 partials
        pgo4 = psum.tile([97, HS], F32, tag="pgo4")      # o gate 4 partials
        pal = psum.tile([C, HS], F32, tag="pal")         # alpha pre-activation
        pgig = psum.tile([1, 512], F32, tag="pgig")      # combined [i|g]
        pgo = psum.tile([1, HS], F32, tag="pgo")         # combined o
        ps0 = psum.tile([1, HS], F32, tag="ps0")
        ps1 = psum.tile([1, HS], F32, tag="ps1")

        # ---- sync ring: xt/bab, alpha_wi half, gates [i|g], o last ----
        nc.sync.dma_start(out=xt_t[:], in_=xt)
        nc.sync.dma_start(out=bab_t[:], in_=bab)
        IG_SIZES = [4, 4, 8, 8, 8]
        ig_starts = [sum(IG_SIZES[:i]) for i in range(len(IG_SIZES))]
        O_SIZES = [8, 8, 8, 8]
        o_starts = [sum(O_SIZES[:i]) for i in range(len(O_SIZES))]
        ig_tiles = []
        for ci, sz in enumerate(IG_SIZES):
            t = wig_pool.tile([128, 8, 2 * HS], FP8, tag="wig")
            nc.sync.dma_start(
                out=t[:, 0:sz, :], in_=wig[:, ig_starts[ci] : ig_starts[ci] + sz, :]
            )
            ig_tiles.append(t)
            if ci == 0:
                # alpha_wi half rides after the first gates tile so the PE
                # gets real work ~3us earlier (two slices: the wi matmuls
                # sit early in the in-order PE queue and must not stall on
                # one big transfer)
                for lo in (0, 8):
                    nc.sync.dma_start(out=wa_t[:, lo : lo + 8, :],
                                      in_=wa[:, lo : lo + 8, :])
        o_tiles = []
        for ci, sz in enumerate(O_SIZES):
            t = wo_pool.tile([128, 8, HS], FP8, tag="wo")
            nc.sync.dma_start(
                out=t[:, 0:sz, :], in_=wo[:, o_starts[ci] : o_starts[ci] + sz, :]
            )
            o_tiles.append(t)

        # ---- scalar ring (slow ~90GB/s, runs concurrently): ct + the
        # alpha_hh half + small tensors; all consumed mid-kernel.  Moving
        # more than ~0.9MB here LOWERS aggregate bandwidth (the rings share
        # the 16 SDMA engines; measured 260 vs 330GB/s at a 1.4MB share).
        nc.scalar.dma_start(out=ct_t[:], in_=ct)
        nc.scalar.dma_start(out=wa_t[:, 16:24, :], in_=wa[:, 16:24, :])
        nc.scalar.dma_start(out=mask4_t[:], in_=mask4)
        nc.scalar.dma_start(out=mg_t[:], in_=cs)
        nc.scalar.dma_start(out=wa_t[:, 24:32, :], in_=wa[:, 24:32, :])
        nc.scalar.dma_start(out=ones_r[:], in_=ones1)

        # ---- PE helpers ----------------------------------------------
        def ig_mms(lo, hi):
            # chunks 0..27 go to the 4-way partials (groups close at 24..27);
            # chunks 28..31 accumulate directly into the combined pgig after
            # the masked combine ran, so the copy+combine overlap them.
            for kk in range(lo, hi):
                ci = max(i for i, s in enumerate(ig_starts) if s <= kk)
                j = kk % 4
                nc.tensor.matmul(
                    pgig4[32 * j : 32 * j + 1, :],
                    lhsT=xt_t[:, kk : kk + 1],
                    rhs=ig_tiles[ci][:, kk - ig_starts[ci], :],
                    start=False,
                    stop=(24 <= kk < 28),
                    tile_position=(0, 32 * j),
                    skip_group_check=True,
                )

        def wi_mms(lo, hi):
            for ko in range(lo, hi):
                j = ko % 4
                nc.tensor.matmul(
                    pgwi4[32 * j : 32 * j + 1, :],
                    lhsT=xt_t[:, KO_A + ko : KO_A + ko + 1],
                    rhs=wa_t[:, ko, :],
                    start=False,
                    stop=(ko >= KO_A - 4),
                    tile_position=(0, 32 * j),
                    skip_group_check=True,
                )

        def ahh_mms(lo, hi):
            for ko in range(lo, hi):
                nc.tensor.matmul(
                    pal[:],
                    lhsT=ct_t[:, ko, :],
                    rhs=wa_t[:, KO_A + ko, :],
                    start=(ko == 0),
                    stop=False,
                )

        def o_mms(lo, hi):
            # chunks 0..27 go to the partials (groups close at 24..27);
            # 28..31 accumulate into the combined pgo so the copy+combine
            # overlap them instead of serializing after the last matmul.
            for kk in range(lo, hi):
                ci = max(i for i, s in enumerate(o_starts) if s <= kk)
                j = kk % 4
                nc.tensor.matmul(
                    pgo4[32 * j : 32 * j + 1, :],
                    lhsT=xt_t[:, kk : kk + 1],
                    rhs=o_tiles[ci][:, kk - o_starts[ci], :],
                    start=False,
                    stop=(24 <= kk < 28),
                    tile_position=(0, 32 * j),
                    skip_group_check=True,
                )

        # ---- PE emission (matches data-arrival order) ----------------
        nc.tensor.matmul(pgig4[:], lhsT=wz_l[:], rhs=wz_r[:],
                         start=True, stop=True, skip_group_check=True)
        nc.tensor.matmul(pgig4[0:1, :], lhsT=ones_b[:], rhs=bab_t[:, 0:512],
                         start=False, stop=False, tile_position=(0, 0),
                         skip_group_check=True)
        nc.tensor.matmul(pgwi4[:], lhsT=wz_l[:], rhs=wz_r[:, 0:HS],
                         start=True, stop=True, skip_group_check=True)
        nc.tensor.matmul(pgo4[:], lhsT=wz_l[:], rhs=wz_r[:, 0:HS],
                         start=True, stop=True, skip_group_check=True)
        nc.tensor.matmul(pgwi4[0:1, :], lhsT=ones_b[:], rhs=bab_t[:, 768:1024],
                         start=False, stop=False, tile_position=(0, 0),
                         skip_group_check=True)
        nc.tensor.matmul(pgo4[0:1, :], lhsT=ones_b[:], rhs=bab_t[:, 512:768],
                         start=False, stop=False, tile_position=(0, 0),
                         skip_group_check=True)
        # extra warm-up matmuls: keep the HAM activity integrator running
        # while the first weight tiles are still in flight (pgig is reset
        # by the combine matmul's start=True later)
        for _ in range(4):
            nc.tensor.matmul(pgig[:], lhsT=wz_l[:, 0:1], rhs=wz_r[:],
                             start=True, stop=True, skip_group_check=True)
        ig_mms(0, 4)
        wi_mms(0, 8)
        ig_mms(4, 8)
        wi_mms(8, KO_A)
        ahh_mms(0, 8)
        ig_mms(8, 16)
        ahh_mms(8, KO_A)

        # wi partials -> SBUF (ACT), masked broadcast-sum into pal
        wi4_t = singles.tile([97, HS], F32R, tag="wi4")
        nc.scalar.copy(out=wi4_t[:], in_=pgwi4[:])
        nc.tensor.matmul(
            pal[:], lhsT=mask4_t[:, 0:C], rhs=wi4_t[:], start=False, stop=True,
        )

        ig_mms(16, 28)

        # [i|g] partials combine (overlaps the last four gates chunks)
        ig4_t = singles.tile([97, 512], F32R, tag="ig4")
        nc.scalar.copy(out=ig4_t[:], in_=pgig4[:])
        nc.tensor.matmul(pgig[:], lhsT=mask4_t[:, 0:1], rhs=ig4_t[:],
                         start=True, stop=False)
        for kk in range(28, KO_G):
            ci = max(i for i, s in enumerate(ig_starts) if s <= kk)
            nc.tensor.matmul(
                pgig[:],
                lhsT=xt_t[:, kk : kk + 1],
                rhs=ig_tiles[ci][:, kk - ig_starts[ci], :],
                start=False,
                stop=(kk == KO_G - 1),
                skip_group_check=True,
            )

        # ---- alpha rows tail (ACT/DVE; overlaps the o matmuls) --------
        tmp_a = singles.tile([C, HS], F32, tag="tmp_a")
        nc.scalar.activation(out=tmp_a[:], in_=pal[:], func=EXP, scale=-INV_S)
        nc.vector.tensor_scalar_add(out=tmp_a[:], in0=tmp_a[:], scalar1=1.0)
        nc.vector.reciprocal_approx_fast(out=tmp_a[:], in_=tmp_a[:])
        nc.scalar.activation(out=ew_t[:], in_=tmp_a[:], func=EXP)
        nc.vector.tensor_mul(out=mg_t[:], in0=mg_t[:], in1=ew_t[:])

        # ---- gates [i|g] tail: one native TANH covers both gates ------
        #   th = [tanh(pre_i/2) | tanh(pre_g)]   (one scale 1/128)
        #   exp(sigmoid(pre_i)) = e^0.5 * exp(0.5*th_i);  g = th_g
        th_t = singles.tile([1, 512], F32, tag="th")
        nc.scalar.activation(out=th_t[:], in_=pgig[:], func=TANH, scale=0.5 * INV_S)
        ew64_t = singles.tile([1, HS], F32, tag="ew64")
        nc.scalar.activation(out=ew64_t[:], in_=th_t[:, 0:HS], func=EXP, scale=0.5)
        mg64_t = singles.tile([1, HS], F32, tag="mg64")
        nc.vector.scalar_tensor_tensor(
            out=mg64_t[:], in0=ew64_t[:], scalar=E_HALF, in1=th_t[:, HS:512],
            op0=mybir.AluOpType.mult, op1=mybir.AluOpType.mult)

        o_mms(0, 24)
        # K=64 reductions over the alpha rows (emitted after their inputs'
        # writers - Tile dependency tracking is program-order-based)
        nc.tensor.matmul(ps0[:], lhsT=ones_r[:], rhs=ew_t[:],
                         start=True, stop=True)
        nc.tensor.matmul(ps1[:], lhsT=ones_r[:], rhs=mg_t[:],
                         start=True, stop=True)
        o_mms(24, 28)

        # o partials combine (overlaps the last four o chunks)
        o4_t = singles.tile([97, HS], F32R, tag="o4")
        nc.scalar.copy(out=o4_t[:], in_=pgo4[:])
        nc.tensor.matmul(pgo[:], lhsT=mask4_t[:, 0:1], rhs=o4_t[:],
                         start=True, stop=False)
        for kk in range(28, KO_G):
            ci = max(i for i, s in enumerate(o_starts) if s <= kk)
            nc.tensor.matmul(
                pgo[:],
                lhsT=xt_t[:, kk : kk + 1],
                rhs=o_tiles[ci][:, kk - o_starts[ci], :],
                start=False,
                stop=(kk == KO_G - 1),
                skip_group_check=True,
            )

        # ---- close the softmax with the i/g row on DVE ----------------
        s0_t = singles.tile([1, HS], F32, tag="s0")
        nc.vector.scalar_tensor_tensor(
            out=s0_t[:], in0=ew64_t[:], scalar=E_HALF, in1=ps0[:],
            op0=mybir.AluOpType.mult, op1=mybir.AluOpType.add)
        s1_t = singles.tile([1, HS], F32, tag="s1")
        nc.vector.tensor_add(out=s1_t[:], in0=ps1[:], in1=mg64_t[:])
        r_t = singles.tile([1, HS], F32, tag="r")
        nc.vector.reciprocal_approx_fast(out=r_t[:], in_=s0_t[:])
        hc_t = singles.tile([1, 2 * HS], F32, tag="hc")
        c1_t = hc_t[:, 0:HS]
        nc.vector.tensor_mul(out=c1_t, in0=s1_t[:], in1=r_t[:])
        nc.sync.dma_start(out=hc[:, 0:HS], in_=c1_t)

        # h1 = tanh(c1) / (1 + exp(-pre_o)); exp/tanh coexist in the ACT
        # table so neither reloads.
        oe_t = singles.tile([1, HS], F32, tag="oe")
        nc.scalar.activation(out=oe_t[:], in_=pgo[:], func=EXP, scale=-INV_S)
        nc.vector.tensor_scalar_add(out=oe_t[:], in0=oe_t[:], scalar1=1.0)
        nc.vector.reciprocal_approx_fast(out=oe_t[:], in_=oe_t[:])
        t4_t = singles.tile([1, HS], F32, tag="t4")
        nc.scalar.activation(out=t4_t[:], in_=c1_t, func=TANH)
        nc.vector.tensor_mul(out=hc_t[:, HS : 2 * HS], in0=oe_t[:], in1=t4_t[:])

        nc.sync.dma_start(out=hc[:, HS : 2 * HS], in_=hc_t[:, HS : 2 * HS])


def _shard_inputs(input_, c_input, h0, c0, weight_ih, weight_hh,
                  alpha_weight_ih, alpha_weight_hh, bias, alpha_bias):
    """Host-side scatter: column-shard the weights over the hidden dim.

    Weights are quantized once to fp8 e3m4 at scale 64 (g columns 128, the
    tanh 2x factor folded in) and pre-tiled to the [ki=128, ko, n] SBUF
    layout; per-core shards are then cheap slices.
    """
    import ml_dtypes
    f32 = np.float32
    bf16 = ml_dtypes.bfloat16
    e3m4 = ml_dtypes.float8_e3m4

    x_comb = np.concatenate([h0[0], input_[0]]).astype(f32)          # (4096,)
    xt = np.ascontiguousarray(x_comb.reshape(KO_G, 128).T).astype(bf16)
    # c_input.T tiled to [ki=128, ko=16, C]
    ct = np.ascontiguousarray(
        c_input.T.reshape(KO_A, 128, C).transpose(1, 0, 2)).astype(bf16)

    w_full = np.concatenate([weight_hh, weight_ih], axis=0).astype(f32)  # (4096, 3H)
    wig_full = np.empty((KG, 2 * H), f32)
    wig_full[:, 0:H] = w_full[:, 0:H] * WSCALE              # i columns
    wig_full[:, H : 2 * H] = w_full[:, 2 * H : 3 * H] * (2.0 * WSCALE)  # g columns
    wo_full = w_full[:, H : 2 * H] * WSCALE                 # o columns
    del w_full
    wig_t = np.ascontiguousarray(
        wig_full.astype(e3m4).reshape(KO_G, 128, 2 * H).transpose(1, 0, 2))
    del wig_full
    wo_t = np.ascontiguousarray(
        wo_full.astype(e3m4).reshape(KO_G, 128, H).transpose(1, 0, 2))
    del wo_full

    wa_full = np.concatenate([alpha_weight_ih, alpha_weight_hh], axis=0) * WSCALE
    wa_t = np.ascontiguousarray(
        wa_full.astype(e3m4).reshape(2 * KO_A, 128, H).transpose(1, 0, 2))
    del wa_full

    bias = np.asarray(bias, f32)
    alpha_bias = np.asarray(alpha_bias, f32)
    c_input = np.asarray(c_input, f32)

    mask4 = np.zeros((97, C), f32)
    mask4[0::32, :] = 1.0

    in_maps = []
    for k in range(NCORES):
        cols = np.s_[k * HS : (k + 1) * HS]
        wig_k = np.ascontiguousarray(np.concatenate(
            [wig_t[:, :, 0 * H + k * HS : 0 * H + (k + 1) * HS],
             wig_t[:, :, 1 * H + k * HS : 1 * H + (k + 1) * HS]], axis=2))
        bab = np.concatenate(
            [bias[0 * H + k * HS : 0 * H + (k + 1) * HS] * WSCALE,
             bias[2 * H + k * HS : 2 * H + (k + 1) * HS] * (2.0 * WSCALE),
             bias[1 * H + k * HS : 1 * H + (k + 1) * HS] * WSCALE,
             alpha_bias[cols] * WSCALE])[None, :].astype(f32)
        in_maps.append({
            "wig": wig_k,
            "wo": np.ascontiguousarray(wo_t[:, :, cols]),
            "wa": np.ascontiguousarray(wa_t[:, :, cols]),
            "bab": bab,
            "cs": np.ascontiguousarray(c_input[:, cols]),
            "ones1": np.ones((C, 1), f32),
            "mask4": mask4,
            "xt": xt,
            "ct": ct,
        })
    return in_maps


def _run(inputs, trace=False):
    global _nc_cache
    if _nc_cache is None:
        _nc_cache = _build_nc()
    nc = _nc_cache
    in_maps = _shard_inputs(**inputs)
    res = run_bass_kernel_spmd(nc, in_maps, core_ids=list(range(NCORES)), trace=trace)
    h1 = np.concatenate(
        [res.results[k]["hc"][:, HS : 2 * HS] for k in range(NCORES)], axis=1)
    c1 = np.concatenate(
        [res.results[k]["hc"][:, 0:HS] for k in range(NCORES)], axis=1)
    return (h1.astype(np.float32), c1.astype(np.float32)), res


def kernel(input_, c_input, h0, c0, weight_ih, weight_hh,
           alpha_weight_ih, alpha_weight_hh, bias, alpha_bias):
    inputs = dict(
        input_=np.asarray(input_, np.float32),
        c_input=np.asarray(c_input, np.float32),
        h0=np.asarray(h0, np.float32),
        c0=np.asarray(c0, np.float32),
        weight_ih=np.asarray(weight_ih, np.float32),
        weight_hh=np.asarray(weight_hh, np.float32),
        alpha_weight_ih=np.asarray(alpha_weight_ih, np.float32),
        alpha_weight_hh=np.asarray(alpha_weight_hh, np.float32),
        bias=np.asarray(bias, np.float32),
        alpha_bias=np.asarray(alpha_bias, np.float32),
    )
    out, _ = _run(inputs)
    return out
